# revision 19
# baseline (speedup 1.0000x reference)
"""Multi-head attention TRN2 kernel (b=4, n=2048, e=768, h=8 heads, d=96).

Sharding: 8 cores = 4 batches x 2 head-groups (4 heads each).
Each core computes, for its (batch, head-group):
    qkv projection (its heads' columns of Wqkv), per-head attention
    (softmax over full n=2048), and a partial output projection
    (its heads' rows of Wproj). Host sums the two partial outputs per
    batch (row-parallel linear unshard) and concatenates batches.

All matmul operands are float32r (full-rate PE, ~1e-4 relative rounding);
PSUM accumulation is fp32. Scores are computed transposed (ET[nk, nq]) so
no on-chip transposes are needed; softmax denominators come from an extra
ones-column appended to V (row 96 of the PV accumulator). exp() skips the
usual max-subtraction: logits/sqrt(e) for this problem are bounded (~|2|),
far from fp32 overflow. Per-head outputs are staged to DRAM in a
[row-block, 384, 128] layout so the output projection runs K=128-packed
(3 matmuls per chunk) with one contiguous reload per row block.
"""

import os

import numpy as np

import concourse.bacc as bacc
import concourse.mybir as mybir
import concourse.tile as tile
from concourse.bass_utils import run_bass_kernel_spmd

B, N, E = 4, 2048, 768
H = 8          # total heads
HL = 4         # heads per core
D = E // H     # 96
DH = D + 1     # 97 (with denominator column)
KB = E // 128  # 6 contraction blocks
NB = N // 128  # 16 row blocks
NC = 8         # cores
EL = HL * D    # 384 local e-dim
SCALE = float(E) ** -0.5

F32 = mybir.dt.float32
F32R = mybir.dt.float32r
AF = mybir.ActivationFunctionType
MULT = mybir.AluOpType.mult
ADD = mybir.AluOpType.add

_COMPILED = None
LAST_EXEC_NS = None
LAST_RESULTS = None


def _device_reset():
    """Recover a wedged NeuronCore (NRT_EXEC_UNIT_UNRECOVERABLE) via axon."""
    try:
        import ctypes
        import time

        import jax

        jax.devices()
        lib = ctypes.CDLL("/opt/axon/libaxon_pjrt.so")
        lib.axon_reset.restype = ctypes.c_int64
        lib.axon_reset()
        time.sleep(3)
    except Exception:
        pass


def _build():
    nc = bacc.Bacc("TRN2", target_bir_lowering=False, debug=False)

    xT_d = nc.dram_tensor("xT", [E, N], F32, kind="ExternalInput")
    wq_d = nc.dram_tensor("wq", [E, EL], F32, kind="ExternalInput")
    wk_d = nc.dram_tensor("wk", [E, EL], F32, kind="ExternalInput")
    wv_d = nc.dram_tensor("wv", [E, HL * DH], F32, kind="ExternalInput")
    bq_d = nc.dram_tensor("bq", [D, HL], F32, kind="ExternalInput")
    bk_d = nc.dram_tensor("bk", [D, HL], F32, kind="ExternalInput")
    bv_d = nc.dram_tensor("bv", [1, HL * DH], F32, kind="ExternalInput")
    wp_d = nc.dram_tensor("wp", [EL, E], F32, kind="ExternalInput")
    bp_d = nc.dram_tensor("bp", [1, E], F32, kind="ExternalInput")
    ones_d = nc.dram_tensor("ones", [1, 128], F32, kind="ExternalInput")
    out_d = nc.dram_tensor("out", [N, E], F32, kind="ExternalOutput")

    # attention outputs staged per output row-block, heads stacked along
    # partitions: ot_d[nb, h*96+dd, i] = OT_h[dd, nb*128+i]
    ot_d = nc.dram_tensor("ot_stage", [NB, EL, 128], F32R)

    with tile.TileContext(nc) as tc:
        with (
            tc.tile_pool(name="const", bufs=1) as cpool,
            tc.tile_pool(name="xt", bufs=1) as xpool,
            tc.tile_pool(name="qk", bufs=2) as qkpool,
            tc.tile_pool(name="vh", bufs=1) as vpool,
            tc.tile_pool(name="pt", bufs=3) as ptpool,
            tc.tile_pool(name="nrm", bufs=2) as npool,
            tc.tile_pool(name="pp", bufs=2, space="PSUM") as pp,
            tc.tile_pool(name="pattn", bufs=1, space="PSUM") as pattn,
        ):
            # ---- constants (DMA order matters: vproj prereqs first) ----
            wv_sb = []
            for kb in range(KB):
                t = cpool.tile([128, HL * DH], F32R, tag=f"wv{kb}")
                nc.gpsimd.dma_start(t[:], wv_d[kb * 128:(kb + 1) * 128, :])
                wv_sb.append(t)
            bv_sb = cpool.tile([1, HL * DH], F32R, tag="bv")
            nc.gpsimd.dma_start(bv_sb[:], bv_d[:])
            ones_sb = cpool.tile([1, 128], F32R, tag="ones")
            nc.gpsimd.dma_start(ones_sb[:], ones_d[:])

            # xT loads, chunked by column so downstream matmuls start early
            xT_sb = []
            for kb in range(KB):
                t = xpool.tile([128, N], F32R, tag=f"xt{kb}")
                xT_sb.append(t)
            for c in range(4):
                for kb in range(KB):
                    nc.gpsimd.dma_start(
                        xT_sb[kb][:, c * 512:(c + 1) * 512],
                        xT_d[kb * 128:(kb + 1) * 128, c * 512:(c + 1) * 512],
                    )
            wq_sb = []
            wk_sb = []
            for kb in range(KB):
                t = cpool.tile([128, EL], F32R, tag=f"wq{kb}")
                nc.gpsimd.dma_start(t[:], wq_d[kb * 128:(kb + 1) * 128, :])
                wq_sb.append(t)
                t = cpool.tile([128, EL], F32R, tag=f"wk{kb}")
                nc.gpsimd.dma_start(t[:], wk_d[kb * 128:(kb + 1) * 128, :])
                wk_sb.append(t)
            wp_sb = []
            for g in range(3):
                t = cpool.tile([128, E], F32R, tag=f"wp{g}")
                nc.gpsimd.dma_start(t[:], wp_d[g * 128:(g + 1) * 128, :])
                wp_sb.append(t)
            bp_sb = cpool.tile([1, E], F32R, tag="bp")
            nc.gpsimd.dma_start(bp_sb[:], bp_d[:])
            bq_sb = cpool.tile([D, HL], F32, tag="bq")
            nc.sync.dma_start(bq_sb[:], bq_d[:])
            bk_sb = cpool.tile([D, HL], F32, tag="bk")
            nc.sync.dma_start(bk_sb[:], bk_d[:])

            # broadcast bias tiles (one K=1 matmul each, reused everywhere)
            bvb_sb = cpool.tile([128, HL * DH], F32, tag="bvb")
            ps = pp.tile([128, 512], F32, tag="pp")
            nc.tensor.matmul(ps[:, 0:HL * DH], ones_sb[:], bv_sb[:], start=True, stop=True)
            nc.vector.tensor_copy(bvb_sb[:], ps[:, 0:HL * DH])
            bpb_sb = cpool.tile([128, E], F32, tag="bpb")

            # ---- V-hat projection: vhat[nb] [128, HL*97] (V + denom column) ----
            vhat = []
            with nc.named_scope("vproj"):
                for nb in range(NB):
                    ps = pp.tile([128, 512], F32, tag="pp")
                    for kb in range(KB):
                        nc.tensor.matmul(
                            ps[:, 0:HL * DH],
                            xT_sb[kb][:, nb * 128:(nb + 1) * 128],
                            wv_sb[kb][:],
                            start=(kb == 0),
                            stop=(kb == KB - 1),
                        )
                    vt = vpool.tile([128, HL * DH], F32R, tag=f"vh{nb}")
                    nc.vector.tensor_tensor(vt[:], ps[:, 0:HL * DH], bvb_sb[:], ADD)
                    vhat.append(vt)

            # ---- per-head: project qT/kT, attention; norm deferred one slot ----
            def emit_norm(job, after=(None, None)):
                h, qh, acc_sb = job
                with nc.named_scope(f"norm{h}_{qh}"):
                    sums = npool.tile([1, 1024], F32, tag="sums", bufs=1)
                    nc.vector.tensor_copy(sums[:], acc_sb[D:DH, :])
                    rec32 = npool.tile([1, 1024], F32, tag="rec32", bufs=1)
                    nc.vector.reciprocal_approx_fast(rec32[:], sums[:])
                    rec = npool.tile([1, 1024], F32R, tag="rec", bufs=1)
                    nc.vector.tensor_copy(rec[:], rec32[:])
                    for j in range(2):
                        c = 2 * qh + j
                        bc = pp.tile([128, 512], F32, tag="pp")
                        nc.tensor.matmul(
                            bc[0:D, :],
                            ones_sb[:, 0:D],
                            rec[:, j * 512:(j + 1) * 512],
                            start=True,
                            stop=True,
                        )
                        ot = npool.tile([D, 512], F32R, tag="ot")
                        nc.vector.tensor_tensor(
                            ot[:], acc_sb[0:D, j * 512:(j + 1) * 512], bc[0:D, :], MULT
                        )
                        # scatter the 4 row-blocks (off the sync queue)
                        for q in range(4):
                            nc.scalar.dma_start(
                                ot_d[c * 4 + q, h * D:(h + 1) * D, :],
                                ot[:, q * 128:(q + 1) * 128],
                            )
                        if after[j] is not None:
                            after[j]()

            def start_qkproj(h):
                with nc.named_scope(f"qkproj{h}"):
                    qT = qkpool.tile([D, N], F32R, tag="qT", name=f"qT{h}")
                    kT = qkpool.tile([D, N], F32R, tag="kT", name=f"kT{h}")
                return (qT, kT)

            def emit_qkproj_chunk(h, tiles, i):
                qT, kT = tiles
                qk, c = divmod(i, 4)
                w_sb, b_sb, dst, sc = [
                    (wq_sb, bq_sb, qT, SCALE),
                    (wk_sb, bk_sb, kT, 1.0),
                ][qk]
                with nc.named_scope(f"qkproj{h}"):
                    ps = pp.tile([128, 512], F32, tag="pp", name=f"psqk{h}_{i}")
                    for kb in range(KB):
                        nc.tensor.matmul(
                            ps[0:D, :],
                            w_sb[kb][:, h * D:(h + 1) * D],
                            xT_sb[kb][:, c * 512:(c + 1) * 512],
                            start=(kb == 0),
                            stop=(kb == KB - 1),
                        )
                    nc.vector.tensor_scalar(
                        dst[:, c * 512:(c + 1) * 512],
                        ps[0:D, :],
                        sc,
                        b_sb[:, h:h + 1],
                        MULT,
                        ADD,
                    )

            def emit_out(nb):
                otn = npool.tile([128, 3, 128], F32R, tag="otn", bufs=4)
                src = ot_d[nb].rearrange("(g p) i -> p g i", p=128)
                eng = nc.sync if nb % 2 == 0 else nc.scalar
                eng.dma_start(otn[:], src)
                po = pattn.tile([128, E], F32, tag="et", bufs=2)
                for off, w in [(0, 512), (512, 256)]:
                    for g in range(3):
                        nc.tensor.matmul(
                            po[:, off:off + w],
                            otn[:, g, :],
                            wp_sb[g][:, off:off + w],
                            start=(g == 0),
                            stop=(g == 2),
                        )
                osb = npool.tile([128, E], F32, tag="osb", bufs=3)
                nc.vector.tensor_tensor(osb[:], po[:], bpb_sb[:], ADD)
                nc.gpsimd.dma_start(out_d[nb * 128:(nb + 1) * 128, :], osb[:])

            pending = None
            tiles = start_qkproj(0)
            for i in range(8):
                emit_qkproj_chunk(0, tiles, i)
            next_tiles = None
            for h in range(HL):
                qT, kT = tiles
                for qh in range(2):
                    with nc.named_scope(f"attn{h}_{qh}"):
                        acc = pattn.tile([DH, 1024], F32, tag="acc")

                        def emit_pv(kbp, pt):
                            for j in range(2):
                                nc.tensor.matmul(
                                    acc[:, j * 512:(j + 1) * 512],
                                    vhat[kbp][:, h * DH:(h + 1) * DH],
                                    pt[:, j * 512:(j + 1) * 512],
                                    start=(kbp == 0),
                                    stop=(kbp == NB - 1),
                                )

                        prev = None
                        for kb in range(NB):
                            et = pattn.tile([128, 1024], F32, tag="et", bufs=2)
                            for j in range(2):
                                c = 2 * qh + j
                                nc.tensor.matmul(
                                    et[:, j * 512:(j + 1) * 512],
                                    kT[:, kb * 128:(kb + 1) * 128],
                                    qT[:, c * 512:(c + 1) * 512],
                                    start=True,
                                    stop=True,
                                )
                            # PV runs one step behind so exp(kb) overlaps it
                            if prev is not None:
                                emit_pv(kb - 1, prev)
                            pt = ptpool.tile([128, 1024], F32R, tag="pt")
                            nc.scalar.activation(pt[:], et[:], AF.Exp)
                            prev = pt
                            if kb == 6 and pending is not None:
                                emit_norm(pending)
                                pending = None
                            if qh == 1 and h + 1 < HL:
                                # interleave next head's projections into the
                                # exp-wait gaps of this attention pass
                                if kb == 0:
                                    next_tiles = start_qkproj(h + 1)
                                if kb % 2 == 1:
                                    emit_qkproj_chunk(h + 1, next_tiles, kb // 2)
                            if qh == 1 and h == HL - 1 and kb >= 9:
                                # fill the last attention pass with early
                                # output-projection blocks (qh=0 data ready)
                                emit_out(kb - 9)
                        emit_pv(NB - 1, prev)
                        acc_sb = npool.tile([DH, 1024], F32, tag="acc_sb")
                        nc.vector.tensor_copy(acc_sb[:, 0:512], acc[:, 0:512])
                        nc.scalar.copy(acc_sb[:, 512:1024], acc[:, 512:1024])
                        pending = (h, qh, acc_sb)
                tiles = next_tiles
                if h == 0:
                    # build the bproj broadcast late (off the critical start)
                    for off, w in [(0, 512), (512, 256)]:
                        ps = pp.tile([128, 512], F32, tag="pp")
                        nc.tensor.matmul(
                            ps[:, 0:w], ones_sb[:], bp_sb[:, off:off + w],
                            start=True, stop=True,
                        )
                        nc.vector.tensor_copy(bpb_sb[:, off:off + w], ps[:, 0:w])

            # ---- output projection out[n, e] = OT^T @ Wp + bp ----
            # nb 0..7 only needs qh=0 data; run them before the last norm's
            # (h3, qh=1) chain so that chain overlaps PE work.

            emit_out(7)
            emit_norm(
                pending,
                after=(
                    lambda: [emit_out(nb) for nb in range(8, 12)],
                    lambda: [emit_out(nb) for nb in range(12, NB)],
                ),
            )

    nc.compile()
    return nc


def _shard(x, Wqkv, bqkv, Wproj, bproj):
    """Build per-core input maps. Core c -> (batch c//2, head-group c%2)."""
    Wr = np.ascontiguousarray(Wqkv.reshape(E, H, D, 3))
    br = np.ascontiguousarray(bqkv.reshape(H, D, 3))
    ones = np.ones((1, 128), np.float32)
    in_maps = []
    for c in range(NC):
        bb, hg = divmod(c, 2)
        hs = slice(hg * HL, (hg + 1) * HL)
        wq = np.ascontiguousarray(Wr[:, hs, :, 0].reshape(E, EL))
        wk = np.ascontiguousarray(Wr[:, hs, :, 1].reshape(E, EL))
        wv = np.zeros((E, HL, DH), np.float32)
        wv[:, :, :D] = Wr[:, hs, :, 2]
        bq = np.ascontiguousarray((br[hs, :, 0] * SCALE).T)  # [D, HL], pre-scaled
        bk = np.ascontiguousarray(br[hs, :, 1].T)
        bv = np.zeros((HL, DH), np.float32)
        bv[:, :D] = br[hs, :, 2]
        bv[:, D] = 1.0  # denominator ones column
        wp = np.ascontiguousarray(Wproj[hg * EL:(hg + 1) * EL, :])
        bp = bproj if hg == 0 else np.zeros_like(bproj)
        in_maps.append({
            "xT": np.ascontiguousarray(x[bb].T),
            "wq": wq,
            "wk": wk,
            "wv": np.ascontiguousarray(wv.reshape(E, HL * DH)),
            "bq": bq,
            "bk": bk,
            "bv": np.ascontiguousarray(bv.reshape(1, HL * DH)),
            "wp": wp,
            "bp": np.ascontiguousarray(bp.reshape(1, E)),
            "ones": ones,
        })
    return in_maps


def kernel(x, Wqkv, bqkv, Wproj, bproj):
    global _COMPILED, LAST_EXEC_NS, LAST_RESULTS
    x = np.asarray(x, dtype=np.float32)
    Wqkv = np.asarray(Wqkv, dtype=np.float32)
    bqkv = np.asarray(bqkv, dtype=np.float32)
    Wproj = np.asarray(Wproj, dtype=np.float32)
    bproj = np.asarray(bproj, dtype=np.float32)

    if _COMPILED is None:
        _COMPILED = _build()
    nc = _COMPILED

    in_maps = _shard(x, Wqkv, bqkv, Wproj, bproj)
    trace = bool(int(os.environ.get("BASS_MHA_TRACE", "0")))
    try:
        res = run_bass_kernel_spmd(nc, in_maps, list(range(NC)), trace=trace)
    except Exception:
        _device_reset()
        res = run_bass_kernel_spmd(nc, in_maps, list(range(NC)), trace=trace)
    LAST_EXEC_NS = res.exec_time_ns
    LAST_RESULTS = res

    out = np.empty((B, N, E), np.float32)
    for bb in range(B):
        out[bb] = res.results[2 * bb]["out"] + res.results[2 * bb + 1]["out"]
    return out
